# revision 1
# baseline (speedup 1.0000x reference)
"""Causal self-attention (B=4, T=2048, C=1024, H=16, D=64) on 8 trn2 NeuronCores.

Sharding: core c handles batch b = c//2 and head group g = c%2 (8 heads each).
Each core computes the qkv projection for its heads, causal flash attention,
and a partial output projection (its heads' rows of w_proj). The host sums the
two partials per batch.

Per-core kernel layout notes:
  - Host supplies x[b].T (so the contraction dim C lands on SBUF partitions),
    w_attn columns for its heads as [q|k|v] blocks, w_proj rows, and causal
    diagonal mask tiles. All bf16; PSUM accumulation fp32.
  - Q^T/K^T are built head-pair-packed: tile [128, T] = head A dims on
    partitions 0-63, head B on 64-127, so the two K=64 QK matmuls of a pair
    land in distinct PE row groups and overlap.
  - S^T tiles ([k, q] layout) get exp on ScalarE with the 1/sqrt(D) scale
    folded in; the softmax denominator falls out of an appended ones-column on
    V (M=65 AV matmul); the division is applied to y^T before the projection.
  - Work is ordered V -> per-pair (QT/KT -> attention) -> projection so the
    scalar/vector engines stream underneath the PE throughout; one PSUM pool
    (tags: av=2 banks, st=3x2 banks) covers every phase with no pool
    boundaries.
"""

import sys

sys.path.insert(0, "/opt/trn_rl_repo")

import numpy as np
import ml_dtypes

from concourse import bacc, mybir
import concourse.bass as bass
from concourse.tile import TileContext
from concourse.bass_utils import run_bass_kernel_spmd

B, T, C, H, D = 4, 2048, 1024, 16, 64
N_CORES = 8
HL = H // 2  # heads per core: 8
CL = HL * D  # local channels per core: 512
BF16 = mybir.dt.bfloat16
F32 = mybir.dt.float32
KT_TILES = C // 128  # 8 contraction tiles for the qkv projection
TT_TILES = T // 128  # 16 token tiles
QB = 4  # q blocks of 512


def build_program(unroll=1, diag_restrict=True, proj_interleave=True, tb_fuse=True):
    """unroll>1 repeats the whole compute body (for steady-state timing)."""
    nc = bacc.Bacc("TRN2", target_bir_lowering=False, debug=False, num_devices=N_CORES)
    xt = nc.dram_tensor("xt", [C, T], BF16, kind="ExternalInput")
    wa = nc.dram_tensor("wa", [C, 3 * CL], BF16, kind="ExternalInput")
    wp = nc.dram_tensor("wp", [CL, C], BF16, kind="ExternalInput")
    msk = nc.dram_tensor("msk", [128, 128], BF16, kind="ExternalInput")
    outp = nc.dram_tensor("outp", [T, C], F32, kind="ExternalOutput")

    with TileContext(nc) as tc:
        with tc.tile_pool(name="resid", bufs=1) as p_r, tc.tile_pool(
            name="ps", bufs=1, space="PSUM"
        ) as p_ps, tc.tile_pool(name="es", bufs=3) as p_es, tc.tile_pool(
            name="sc", bufs=4
        ) as p_sc, tc.tile_pool(name="ob", bufs=3) as p_ob:
            xt_sb = [p_r.tile([128, T], BF16, name=f"xt{i}", tag=f"xt{i}") for i in range(KT_TILES)]
            wa_sb = [p_r.tile([128, 3 * CL], BF16, name=f"wa{i}", tag=f"wa{i}") for i in range(KT_TILES)]
            wp_sb = [p_r.tile([128, C], BF16, name=f"wp{i}", tag=f"wp{i}") for i in range(4)]
            msk_sb = p_r.tile([128, 128], BF16, tag="msk")
            ones_sb = p_r.tile([128, 64], BF16, tag="ones")
            qt_sb = [p_r.tile([128, T], BF16, name=f"qt{p}", tag=f"qt{p}") for p in range(4)]
            kt_sb = [p_r.tile([128, T], BF16, name=f"kt{p}", tag=f"kt{p}") for p in range(4)]
            va_sb = [p_r.tile([128, HL * 65], BF16, name=f"va{i}", tag=f"va{i}") for i in range(TT_TILES)]
            yt_sb = [p_r.tile([128, T], BF16, name=f"yt{p}", tag=f"yt{p}") for p in range(4)]

            nc.sync.dma_start(out=msk_sb[:], in_=msk[:])
            nc.gpsimd.memset(ones_sb[:], 1.0)

            # HAM warmup: the PE clock-gate sits at 1.2 GHz until ~3.4us of
            # sustained activity. The input DMA ramp leaves the PE idle for
            # ~14us at kernel start, so burn it with dummy matmuls on a
            # memset tile (borrowing an "av" PSUM slot, which real work
            # doesn't need until the first attention block).
            warm = p_r.tile([128, 512], BF16, tag="warm")
            nc.gpsimd.memset(warm[:], 0.0)
            wps = p_ps.tile([128, 512], F32, tag="av", name="warmps", bufs=2)
            for _ in range(18):
                nc.tensor.matmul(
                    wps[:], lhsT=warm[:, 0:128], rhs=warm[:], start=True, stop=True
                )
            for i in range(KT_TILES):
                nc.sync.dma_start(out=xt_sb[i][:], in_=xt[i * 128 : (i + 1) * 128, :])
                nc.sync.dma_start(
                    out=wa_sb[i][:, 2 * CL : 3 * CL],
                    in_=wa[i * 128 : (i + 1) * 128, 2 * CL : 3 * CL],
                )
            for i in range(KT_TILES):
                nc.sync.dma_start(
                    out=wa_sb[i][:, 0 : 2 * CL], in_=wa[i * 128 : (i + 1) * 128, 0 : 2 * CL]
                )
            for i in range(4):
                nc.sync.dma_start(out=wp_sb[i][:], in_=wp[i * 128 : (i + 1) * 128, :])

            for _ in range(unroll):
                # ---- closure builders; each closure is a chunk of PE work
                # that can be interleaved into the attention kt loops so the
                # PE static order never runs dry while ScalarE paces exp.
                def v_closures(tt):
                    state = {}

                    def h1():
                        acc = p_ps.tile([128, 512], F32, tag="fill", name="accv", bufs=2)
                        state["acc"] = acc
                        for ki in range(4):
                            nc.tensor.matmul(
                                acc[:],
                                lhsT=xt_sb[ki][:, tt * 128 : (tt + 1) * 128],
                                rhs=wa_sb[ki][:, 2 * CL : 3 * CL],
                                start=(ki == 0),
                                stop=False,
                            )

                    def h2():
                        acc = state["acc"]
                        for ki in range(4, KT_TILES):
                            nc.tensor.matmul(
                                acc[:],
                                lhsT=xt_sb[ki][:, tt * 128 : (tt + 1) * 128],
                                rhs=wa_sb[ki][:, 2 * CL : 3 * CL],
                                start=False,
                                stop=(ki == KT_TILES - 1),
                            )
                        va_view = va_sb[tt].rearrange("p (h e) -> p h e", e=65)
                        nc.scalar.copy(
                            out=va_view[:, :, 0:64],
                            in_=acc[:].rearrange("p (h e) -> p h e", e=64),
                        )
                        nc.gpsimd.memset(va_view[:, :, 64:65], 1.0)

                    return [h1, h2]

                def accqk_closures(p, qb):
                    out = []
                    for dst, col0 in ((qt_sb, 0), (kt_sb, CL)):
                        state = {}

                        def h1(dst=dst, col0=col0, state=state):
                            acc = p_ps.tile([128, 512], F32, tag="fill", name="accqk", bufs=2)
                            state["acc"] = acc
                            for ki in range(4):
                                nc.tensor.matmul(
                                    acc[:],
                                    lhsT=wa_sb[ki][:, col0 + p * 128 : col0 + (p + 1) * 128],
                                    rhs=xt_sb[ki][:, qb * 512 : (qb + 1) * 512],
                                    start=(ki == 0),
                                    stop=False,
                                )

                        def h2(dst=dst, col0=col0, state=state):
                            acc = state["acc"]
                            for ki in range(4, KT_TILES):
                                nc.tensor.matmul(
                                    acc[:],
                                    lhsT=wa_sb[ki][:, col0 + p * 128 : col0 + (p + 1) * 128],
                                    rhs=xt_sb[ki][:, qb * 512 : (qb + 1) * 512],
                                    start=False,
                                    stop=(ki == KT_TILES - 1),
                                )
                            nc.scalar.copy(
                                out=dst[p][:, qb * 512 : (qb + 1) * 512], in_=acc[:]
                            )

                        out += [h1, h2]
                    return out

                def proj_closures(tt, split=False):
                    # split=True: first closures accumulate pairs 0-2 (no
                    # dependency on the last pair's division), the rest add
                    # pair 3 and store.
                    state = {}

                    def mk(nb, pps, last):
                        def h():
                            if nb == 0 and pps[0] == 0:
                                state["ob"] = p_ob.tile([128, C], F32, tag="ob", name="ob")
                            if pps[0] == 0:
                                state[nb] = p_ps.tile(
                                    [128, 512], F32, tag="fill", name="accp", bufs=2
                                )
                            acc = state[nb]
                            for pp in pps:
                                nc.tensor.matmul(
                                    acc[:],
                                    lhsT=yt_sb[pp][:, tt * 128 : (tt + 1) * 128],
                                    rhs=wp_sb[pp][:, nb * 512 : (nb + 1) * 512],
                                    start=(pp == 0),
                                    stop=(pp == 3),
                                )
                            if not last:
                                return
                            ob = state["ob"]
                            nc.vector.tensor_copy(
                                out=ob[:, nb * 512 : (nb + 1) * 512], in_=acc[:]
                            )
                            nc.sync.dma_start(
                                out=outp[
                                    tt * 128 : (tt + 1) * 128, nb * 512 : (nb + 1) * 512
                                ],
                                in_=ob[:, nb * 512 : (nb + 1) * 512],
                            )

                        return h

                    if not split:
                        return [mk(0, range(4), True), mk(1, range(4), True)]
                    return (
                        [mk(0, range(3), False), mk(1, range(3), False)],
                        [mk(0, [3], True), mk(1, [3], True)],
                    )

                blocks = [(p, qb) for p in range(4) for qb in range(QB)]

                # prologue: V tiles for q block 0 and pair 0's first QT/KT
                for tt in range(4):
                    for cl in v_closures(tt):
                        cl()
                for cl in accqk_closures(0, 0):
                    cl()

                last_final_tails = []
                for idx, (p, qb) in enumerate(blocks):
                    hA, hB = 2 * p, 2 * p + 1
                    filler = []
                    if idx + 1 < len(blocks):
                        filler += accqk_closures(*blocks[idx + 1])
                    if p == 0 and qb < QB - 1:
                        for tt in range(4 * (qb + 1), 4 * (qb + 1) + 4):
                            filler += v_closures(tt)
                    if p == 3 and qb > 0 and proj_interleave:
                        for tt in range(4 * (qb - 1), 4 * qb):
                            filler += proj_closures(tt)


                    filler_all = list(filler)
                    nkt = 4 * qb + 4
                    LAG = 3  # AV trails QK by this many k tiles
                    ya = p_ps.tile([128, 512], F32, tag="av", name="ya", bufs=2)
                    yb = p_ps.tile([128, 512], F32, tag="av", name="yb", bufs=2)
                    es_ring = {}

                    def emit_qk(kt, p=p, qb=qb):
                        # Diagonal tiles only need q columns >= kt*128; the
                        # causal triangle then only lives in the first 128 of
                        # the remaining columns. st/es keep head B at column
                        # 512 (fixed full-size tiles).
                        j = kt - 4 * qb
                        qoff = max(j, 0) * 128 if diag_restrict else 0
                        W = 512 - qoff
                        st = p_ps.tile([128, 1024], F32, tag="st", name="st", bufs=2)
                        nc.tensor.matmul(
                            st[:, 0:W],
                            lhsT=kt_sb[p][0:64, kt * 128 : (kt + 1) * 128],
                            rhs=qt_sb[p][0:64, qb * 512 + qoff : (qb + 1) * 512],
                            start=True,
                            stop=True,
                        )
                        nc.tensor.matmul(
                            st[:, 512 : 512 + W],
                            lhsT=kt_sb[p][64:128, kt * 128 : (kt + 1) * 128],
                            rhs=qt_sb[p][64:128, qb * 512 + qoff : (qb + 1) * 512],
                            start=True,
                            stop=True,
                        )
                        es = p_es.tile([128, 1024], BF16, tag="es", name="es", bufs=6)
                        if W == 512:
                            nc.scalar.activation(
                                out=es[:],
                                in_=st[:],
                                func=mybir.ActivationFunctionType.Exp,
                                scale=0.125,
                            )
                        else:
                            view_es = es.rearrange("p (h w) -> p h w", h=2)[:, :, 0:W]
                            view_st = st.rearrange("p (h w) -> p h w", h=2)[:, :, 0:W]
                            nc.scalar.activation(
                                out=view_es,
                                in_=view_st,
                                func=mybir.ActivationFunctionType.Exp,
                                scale=0.125,
                            )
                        if j >= 0:
                            nc.vector.tensor_tensor(
                                es[:, 0:128], es[:, 0:128], msk_sb[:], mybir.AluOpType.mult
                            )
                            nc.vector.tensor_tensor(
                                es[:, 512:640], es[:, 512:640], msk_sb[:], mybir.AluOpType.mult
                            )
                        es_ring[kt] = (es, qoff, W)

                    def emit_av(kt, p=p, qb=qb, nkt=nkt, ya=ya, yb=yb):
                        es, qoff, W = es_ring.pop(kt)
                        nc.tensor.matmul(
                            ya[0:65, qoff:512],
                            lhsT=va_sb[kt][:, hA * 65 : (hA + 1) * 65],
                            rhs=es[:, 0:W],
                            start=(kt == 0),
                            stop=(kt == nkt - 1),
                        )
                        nc.tensor.matmul(
                            yb[0:65, qoff:512],
                            lhsT=va_sb[kt][:, hB * 65 : (hB + 1) * 65],
                            rhs=es[:, 512 : 512 + W],
                            start=(kt == 0),
                            stop=(kt == nkt - 1),
                        )

                    total_iters = nkt + LAG
                    pops_done = 0
                    for kt in range(total_iters):
                        if kt < nkt:
                            emit_qk(kt)
                        if kt >= LAG:
                            emit_av(kt - LAG)
                        want = (kt + 1) * len(filler_all) // total_iters
                        while pops_done < want and filler:
                            filler.pop(0)()
                            pops_done += 1
                    while filler:
                        filler.pop(0)()

                    div_order = ((1, yb), (0, ya)) if idx == len(blocks) - 1 else ((0, ya), (1, yb))
                    for s, yy in div_order:
                        rec = p_sc.tile([128, 512], BF16, tag="rec", name="rec")
                        with nc.allow_low_precision(reason="softmax denom in bf16"):
                            nc.vector.reciprocal(out=rec[64:65, :], in_=yy[64:65, :])
                        rep = p_ps.tile([128, 512], F32, tag="fill", name="rep", bufs=2)
                        nc.tensor.matmul(
                            rep[0:64, :],
                            lhsT=ones_sb[64:65, 0:64],
                            rhs=rec[64:65, :],
                            start=True,
                            stop=True,
                        )
                        reps = p_sc.tile([128, 512], BF16, tag="reps", name="reps")
                        nc.vector.tensor_copy(out=reps[0:64, :], in_=rep[0:64, :])
                        if s == 0:
                            nc.vector.tensor_tensor(
                                yt_sb[p][0:64, qb * 512 : (qb + 1) * 512],
                                yy[0:64, :],
                                reps[0:64, :],
                                mybir.AluOpType.mult,
                            )
                        else:
                            tmp = p_sc.tile([128, 512], BF16, tag="ytmp", name="ytmp")
                            nc.vector.tensor_tensor(
                                tmp[0:64, :],
                                yy[0:64, :],
                                reps[0:64, :],
                                mybir.AluOpType.mult,
                            )
                            nc.sync.dma_start(
                                out=yt_sb[p][64:128, qb * 512 : (qb + 1) * 512],
                                in_=tmp[0:64, :],
                            )

                # final projection group (last q block of the last pair)
                for tt in range(4 * (QB - 1), 4 * QB):
                    for cl in proj_closures(tt):
                        cl()

    nc.compile()
    return nc


_NC_CACHE = None


def _get_program():
    global _NC_CACHE
    if _NC_CACHE is None:
        _NC_CACHE = build_program()
    return _NC_CACHE


def _make_masks():
    ki = np.arange(128)[:, None]
    qi = np.arange(128)[None, :]
    return (ki <= qi).astype(np.float32).astype(ml_dtypes.bfloat16)


def make_in_maps(x, w_attn, w_proj):
    msk = _make_masks()
    xts = [x[b].T.astype(ml_dtypes.bfloat16, order="C") for b in range(B)]
    was, wps = [], []
    for g in range(2):
        was.append(
            np.concatenate(
                [
                    w_attn[:, 512 * g : 512 * g + 512],
                    w_attn[:, C + 512 * g : C + 512 * g + 512],
                    w_attn[:, 2 * C + 512 * g : 2 * C + 512 * g + 512],
                ],
                axis=1,
            ).astype(ml_dtypes.bfloat16, order="C")
        )
        wps.append(
            w_proj[512 * g : 512 * g + 512, :].astype(ml_dtypes.bfloat16, order="C")
        )
    return [
        {"xt": xts[c // 2], "wa": was[c % 2], "wp": wps[c % 2], "msk": msk}
        for c in range(N_CORES)
    ]


def gather_output(results):
    out = np.empty((B, T, C), np.float32)
    for b in range(B):
        out[b] = results[2 * b]["outp"] + results[2 * b + 1]["outp"]
    return out


_RUNNER = None


def _make_cached_runner(nc):
    """jit the SPMD executable once so repeat kernel() calls skip the
    per-call retrace/recompile that run_bass_kernel_spmd pays."""
    import jax
    from jax.sharding import Mesh, PartitionSpec
    from jax.experimental.shard_map import shard_map
    from concourse import bass2jax

    bass2jax.install_neuronx_cc_hook()
    partition_name = nc.partition_id_tensor.name if nc.partition_id_tensor else None
    in_names, out_names, out_avals, zero_outs = [], [], [], []
    for alloc in nc.m.functions[0].allocations:
        if not isinstance(alloc, mybir.MemoryLocationSet):
            continue
        name = alloc.memorylocations[0].name
        if alloc.kind == "ExternalInput":
            if name != partition_name:
                in_names.append(name)
        elif alloc.kind == "ExternalOutput":
            shape = tuple(alloc.tensor_shape)
            dtype = mybir.dt.np(alloc.dtype)
            out_names.append(name)
            out_avals.append(jax.core.ShapedArray(shape, dtype))
            zero_outs.append(np.zeros(shape, dtype))
    n_params = len(in_names)
    n_outs = len(out_avals)
    all_in_names = in_names + out_names
    if partition_name is not None:
        all_in_names.append(partition_name)

    def _body(*args):
        operands = list(args)
        if partition_name is not None:
            operands.append(bass2jax.partition_id_tensor())
        return tuple(
            bass2jax._bass_exec_p.bind(
                *operands,
                out_avals=tuple(out_avals),
                in_names=tuple(all_in_names),
                out_names=tuple(out_names),
                lowering_input_output_aliases=(),
                sim_require_finite=True,
                sim_require_nnan=True,
                nc=nc,
            )
        )

    devices = jax.devices()[:N_CORES]
    mesh = Mesh(np.asarray(devices), ("core",))
    spec = PartitionSpec("core")
    sharded = jax.jit(
        shard_map(
            _body,
            mesh=mesh,
            in_specs=(spec,) * (n_params + n_outs),
            out_specs=(spec,) * n_outs,
            check_rep=False,
        ),
        donate_argnums=tuple(range(n_params, n_params + n_outs)),
        keep_unused=True,
    )

    def run(in_maps):
        concat_in = [
            np.concatenate([np.asarray(in_maps[c][nm]) for c in range(N_CORES)], 0)
            for nm in in_names
        ]
        zeros = [
            np.zeros((N_CORES * z.shape[0], *z.shape[1:]), z.dtype) for z in zero_outs
        ]
        outs = sharded(*concat_in, *zeros)
        return [
            {
                name: np.asarray(outs[i]).reshape(N_CORES, *out_avals[i].shape)[c]
                for i, name in enumerate(out_names)
            }
            for c in range(N_CORES)
        ]

    return run


def kernel(x, w_attn, w_proj):
    global _RUNNER
    x = np.asarray(x, dtype=np.float32)
    w_attn = np.asarray(w_attn, dtype=np.float32)
    w_proj = np.asarray(w_proj, dtype=np.float32)
    nc = _get_program()
    in_maps = make_in_maps(x, w_attn, w_proj)
    if _RUNNER is None:
        try:
            _RUNNER = _make_cached_runner(nc)
        except Exception:
            _RUNNER = None
        if _RUNNER is None:
            res = run_bass_kernel_spmd(nc, in_maps, core_ids=list(range(N_CORES)))
            return gather_output(res.results)
    try:
        return gather_output(_RUNNER(in_maps))
    except Exception:
        res = run_bass_kernel_spmd(nc, in_maps, core_ids=list(range(N_CORES)))
        return gather_output(res.results)



# revision 41
# speedup vs baseline: 1.1130x; 1.1130x over previous
"""Causal self-attention (B=4, T=2048, C=1024, H=16, D=64) on 8 trn2 NeuronCores.

Sharding: core c handles batch b = c//2 and head group g = c%2 (8 heads each).
Each core computes the qkv projection for its heads, causal flash attention,
and a partial output projection (its heads' rows of w_proj). The host sums the
two partials per batch.

Per-core kernel layout notes:
  - Host supplies x[b].T (so the contraction dim C lands on SBUF partitions),
    w_attn columns for its heads laid out as [qk_p0|qk_p1|qk_p2|qk_p3|v]
    (per-pair 128 q cols + 128 k cols, then the 512 v cols), w_proj rows, a
    causal diagonal mask tile, and a 128x128 identity (for PE transposes).
    All bf16; PSUM accumulation fp32.
  - Q^T/K^T are head-pair-packed: tile [128, T] = head A dims on partitions
    0-63, head B on 64-127, so the two K=64 QK matmuls of a pair use distinct
    PE row groups.
  - S^T tiles ([k, q] layout) get exp on ScalarE with the 1/sqrt(D) scale
    folded in. AV runs with the exp'd S^T chunk as the *stationary* operand
    (lhsT = es [128k, 128q], rhs = v [128k, 64]), so each AV matmul streams
    only 64 columns instead of up to 512 - half the PE time of the
    v-stationary form. A parallel N=1 matmul against a ones column
    accumulates the softmax denominator.
  - The denominator lands per-partition (q on partitions), so the softmax
    division is a native DVE tensor_scalar with a per-partition reciprocal -
    no PE broadcast matmuls. The normalized y [q, d] tiles are transposed
    back to [d, q] by the DMA crossbar (dma_start_transpose), which is nearly
    free on the otherwise-idle DMA engines.
  - Work is ordered V -> per-pair (QT/KT -> attention) -> projection, with
    all projection work interleaved into the attention kt loops as filler so
    the PE static order never runs dry while ScalarE paces exp.
  - PSUM is bank-granular: st ring 2x2 banks, yacc ring 2x1, one fill bank
    (manually double-buffered halves for the projection accumulators) and one
    denominator bank.
"""

import sys

sys.path.insert(0, "/opt/trn_rl_repo")

import numpy as np
import ml_dtypes

from concourse import bacc, mybir
import concourse.bass as bass
from concourse.tile import TileContext
from concourse.bass_utils import run_bass_kernel_spmd

B, T, C, H, D = 4, 2048, 1024, 16, 64
N_CORES = 8
HL = H // 2  # heads per core: 8
CL = HL * D  # local channels per core: 512
BF16 = mybir.dt.bfloat16
F32 = mybir.dt.float32
KT_TILES = C // 128  # 8 contraction tiles for the qkv projection
TT_TILES = T // 128  # 16 token tiles
QB = 4  # q blocks of 512


EMIT_LOG = []  # label per PE matmul emission, for trace attribution


def _log(label):
    EMIT_LOG.append(label)


def build_program(unroll=1, warm=18):
    """unroll>1 repeats the whole compute body (for steady-state timing)."""
    EMIT_LOG.clear()
    nc = bacc.Bacc("TRN2", target_bir_lowering=False, debug=False, num_devices=N_CORES)
    # Packed partition-major inputs (one SBUF-shaped row per partition) so
    # the whole input load is a handful of large byte-bound DMAs instead of
    # ~45 dispatches on the serial HWDGE queue.
    #   xt: col = qb*4096 + ki*512 + t      (token block qb, C-chunk ki)
    #   wa: front 8x768 = per-ki [v 512 | q0 128 | k0 128], then
    #       back  8x768 = per-ki [q1 128 | k1 128 | ... | q3 | k3]
    #   wp: col = ki*1024 + c
    xt = nc.dram_tensor("xt", [128, 4 * KT_TILES * 512], BF16, kind="ExternalInput")
    wa = nc.dram_tensor("wa", [128, KT_TILES * 1536], BF16, kind="ExternalInput")
    wp = nc.dram_tensor("wp", [128, 4 * 1024], BF16, kind="ExternalInput")
    msk = nc.dram_tensor("msk", [128, 128], BF16, kind="ExternalInput")
    idn = nc.dram_tensor("idn", [128, 128], F32, kind="ExternalInput")
    outp = nc.dram_tensor("outp", [T, C], F32, kind="ExternalOutput")

    with TileContext(nc) as tc:
        with tc.tile_pool(name="resid", bufs=1) as p_r, tc.tile_pool(
            name="ps", bufs=1, space="PSUM"
        ) as p_ps, tc.tile_pool(name="es", bufs=3) as p_es, tc.tile_pool(
            name="sc", bufs=4
        ) as p_sc, tc.tile_pool(name="ob", bufs=4) as p_ob:
            xt_sb = p_r.tile([128, 4 * KT_TILES * 512], BF16, name="xts", tag="xts")
            wa_sb = p_r.tile([128, KT_TILES * 1536], BF16, name="was", tag="was")
            wp_sb = p_r.tile([128, 4 * 1024], BF16, name="wps", tag="wps")

            def xt_ap(ki, qb, a, b):
                # token columns [qb*512+a, qb*512+b) of C-chunk ki
                return xt_sb[:, qb * 4096 + ki * 512 + a : qb * 4096 + ki * 512 + b]

            def wav_ap(ki, a, b):
                return wa_sb[:, ki * 768 + a : ki * 768 + b]

            def waqk_ap(ki, p, k_not_q):
                if p == 0:
                    c0 = ki * 768 + 512 + k_not_q * 128
                else:
                    c0 = 6144 + ki * 768 + (p - 1) * 256 + k_not_q * 128
                return wa_sb[:, c0 : c0 + 128]
            msk_sb = p_r.tile([128, 128], BF16, tag="msk")
            idn_sb = p_r.tile([128, 128], F32, tag="idn")
            ones_sb = p_r.tile([128, 1], BF16, tag="ones")
            qt_sb = [p_r.tile([128, T], BF16, name=f"qt{p}", tag=f"qt{p}") for p in range(4)]
            kt_sb = [p_r.tile([128, T], BF16, name=f"kt{p}", tag=f"kt{p}") for p in range(4)]
            va_sb = [p_r.tile([128, HL * 65], BF16, name=f"va{i}", tag=f"va{i}") for i in range(TT_TILES)]
            yt_sb = [p_r.tile([128, T], BF16, name=f"yt{p}", tag=f"yt{p}") for p in range(4)]

            nc.sync.dma_start(out=msk_sb[:], in_=msk[:])
            nc.gpsimd.memset(ones_sb[:], 1.0)

            # HAM warmup: the PE clock-gate sits at 1.2 GHz until ~3us of
            # sustained activity. The input DMA ramp leaves the PE idle at
            # kernel start, so burn it with dummy matmuls on a memset tile
            # (borrowing a "yacc" PSUM slot, which real work doesn't need
            # until the first attention block).
            warmt = p_r.tile([128, 512], BF16, tag="warm")
            nc.gpsimd.memset(warmt[:], 0.0)
            wps = p_ps.tile([128, 1024], F32, tag="st", name="warmps", bufs=2)
            for _ in range(warm):
                _log("warm")
                nc.tensor.matmul(
                    wps[:, 0:512], lhsT=warmt[:, 0:128], rhs=warmt[:], start=True, stop=True
                )
            # PSUM is bank-granular (8 x 2KB): st ring 2x2 banks, fill ring
            # 2x1, and one wide single-buffered yacc (2 banks) holding the
            # per-chunk [y_A 65 | y_B 65] AV accumulators, denominator riding
            # in column 64 via the ones column interleaved into va.
            yacc_sb = p_ps.tile([128, 1024], F32, tag="yacc", name="yacc", bufs=1)

            def fill_slot():
                return p_ps.tile([128, 512], F32, tag="fill", name="fill", bufs=2)[:]

            # ones columns of every va tile (written once; the v-projection
            # copies write through a 65-stride view that skips them)
            for tt in range(TT_TILES):
                va_view = va_sb[tt].rearrange("p (h e) -> p h e", e=65)
                nc.gpsimd.memset(va_view[:, :, 64:65], 1.0)
            # Input DMAs, ordered so the prologue (V tiles 0-3 + pair-0
            # QT/KT for q block 0) is fed first, in a handful of large
            # transfers: wa front (v + pair-0 qk) and xt token-block 0 split
            # in two for finer chasing, then the remaining xt blocks, the
            # other pairs' qk columns, and wp.
            for h in range(2):
                nc.sync.dma_start(
                    out=wa_sb[:, h * 3072 : (h + 1) * 3072],
                    in_=wa[:, h * 3072 : (h + 1) * 3072],
                )
                nc.sync.dma_start(
                    out=xt_sb[:, h * 2048 : (h + 1) * 2048],
                    in_=xt[:, h * 2048 : (h + 1) * 2048],
                )
            for qb in range(1, QB):
                nc.sync.dma_start(
                    out=xt_sb[:, qb * 4096 : (qb + 1) * 4096],
                    in_=xt[:, qb * 4096 : (qb + 1) * 4096],
                )
            nc.sync.dma_start(out=wa_sb[:, 6144:12288], in_=wa[:, 6144:12288])
            nc.sync.dma_start(out=wp_sb[:], in_=wp[:])
            nc.sync.dma_start(out=idn_sb[:], in_=idn[:])

            for _ in range(unroll):
                # ---- closure builders; each closure is a chunk of PE work
                # that can be interleaved into the attention kt loops so the
                # PE static order never runs dry while ScalarE paces exp.
                def v_closures(tt):
                    def h(tt=tt):
                        acc = fill_slot()
                        for ki in range(KT_TILES):
                            _log(f"v[{tt}]k{ki}")
                            nc.tensor.matmul(
                                acc,
                                lhsT=xt_ap(ki, tt // 4, (tt % 4) * 128, (tt % 4) * 128 + 128),
                                rhs=wav_ap(ki, 0, 512),
                                start=(ki == 0),
                                stop=(ki == KT_TILES - 1),
                            )
                        va_view = va_sb[tt].rearrange("p (h e) -> p h e", e=65)
                        nc.vector.tensor_copy(
                            out=va_view[:, :, 0:64],
                            in_=acc.rearrange("p (h e) -> p h e", e=64),
                        )
                    return [h]

                def accqk_closures(p, qb):
                    out = []
                    for dst, knq in ((qt_sb, 0), (kt_sb, 1)):
                        def h(dst=dst, knq=knq, p=p, qb=qb):
                            acc = fill_slot()
                            for ki in range(KT_TILES):
                                _log(f"qk[{p},{qb},{knq}]k{ki}")
                                nc.tensor.matmul(
                                    acc,
                                    lhsT=waqk_ap(ki, p, knq),
                                    rhs=xt_ap(ki, qb, 0, 512),
                                    start=(ki == 0),
                                    stop=(ki == KT_TILES - 1),
                                )
                            nc.vector.tensor_copy(
                                out=dst[p][:, qb * 512 : (qb + 1) * 512],
                                in_=acc,
                            )
                        out.append(h)
                    return out

                def proj_closures(tt):
                    out = []
                    state = {}

                    def get_ob():
                        if "ob" not in state:
                            state["ob"] = p_ob.tile([128, C], F32, tag="ob", name="ob")
                        return state["ob"]

                    for nb in range(2):
                        def h(tt=tt, nb=nb):
                            acc = fill_slot()
                            for pp in range(4):
                                _log(f"proj[{tt},{nb}]p{pp}")
                                nc.tensor.matmul(
                                    acc,
                                    lhsT=yt_sb[pp][:, tt * 128 : (tt + 1) * 128],
                                    rhs=wp_sb[:, pp * 1024 + nb * 512 : pp * 1024 + (nb + 1) * 512],
                                    start=(pp == 0),
                                    stop=(pp == 3),
                                )
                            ob = get_ob()
                            nc.vector.tensor_copy(
                                out=ob[:, nb * 512 : (nb + 1) * 512], in_=acc
                            )
                            if nb == 1:
                                nc.sync.dma_start(
                                    out=outp[tt * 128 : (tt + 1) * 128, :], in_=ob[:]
                                )
                        out.append(h)
                    return out

                blocks = [(p, qb) for p in range(4) for qb in range(QB)]

                # prologue: V tiles for q block 0 and pair 0's first QT/KT
                for tt in range(4):
                    for cl in v_closures(tt):
                        cl()
                for cl in accqk_closures(0, 0):
                    cl()

                for idx, (p, qb) in enumerate(blocks):
                    filler = []
                    filler_b = []
                    if idx + 1 < len(blocks):
                        filler += accqk_closures(*blocks[idx + 1])
                    if p == 0 and qb < QB - 1:
                        for tt in range(4 * (qb + 1), 4 * (qb + 1) + 4):
                            filler += v_closures(tt)
                    if p == 3 and qb > 0:
                        # proj consumes the previous block's transposes; pop it
                        # only in the second half of the block so the DMA
                        # crossbar has drained by then.
                        for tt in range(4 * (qb - 1), 4 * qb):
                            filler_b += proj_closures(tt)

                    nkt = 4 * qb + 4
                    LAG = 3  # AV trails QK by this many k tiles
                    es_ring = {}

                    def emit_qk(kt, p=p, qb=qb):
                        # Diagonal tiles only need q columns >= kt*128; the
                        # causal triangle then only lives in the first 128 of
                        # the remaining columns. st/es keep head B at column
                        # 512 (fixed full-size tiles).
                        j = kt - 4 * qb
                        qoff = max(j, 0) * 128
                        W = 512 - qoff
                        st = p_ps.tile([128, 1024], F32, tag="st", name="st", bufs=2)
                        _log(f"QK[{p},{qb}]kt{kt}A")
                        nc.tensor.matmul(
                            st[:, 0:W],
                            lhsT=kt_sb[p][0:64, kt * 128 : (kt + 1) * 128],
                            rhs=qt_sb[p][0:64, qb * 512 + qoff : (qb + 1) * 512],
                            start=True,
                            stop=True,
                        )
                        _log(f"QK[{p},{qb}]kt{kt}B")
                        nc.tensor.matmul(
                            st[:, 512 : 512 + W],
                            lhsT=kt_sb[p][64:128, kt * 128 : (kt + 1) * 128],
                            rhs=qt_sb[p][64:128, qb * 512 + qoff : (qb + 1) * 512],
                            start=True,
                            stop=True,
                        )
                        es = p_es.tile([128, 1024], BF16, tag="es", name="es", bufs=6)
                        if W == 512:
                            nc.scalar.activation(
                                out=es[:],
                                in_=st[:],
                                func=mybir.ActivationFunctionType.Exp,
                                scale=0.125,
                            )
                        else:
                            view_es = es.rearrange("p (h w) -> p h w", h=2)[:, :, 0:W]
                            view_st = st.rearrange("p (h w) -> p h w", h=2)[:, :, 0:W]
                            nc.scalar.activation(
                                out=view_es,
                                in_=view_st,
                                func=mybir.ActivationFunctionType.Exp,
                                scale=0.125,
                            )
                        if j >= 0:
                            nc.gpsimd.tensor_tensor(
                                es[:, 0:128], es[:, 0:128], msk_sb[:], mybir.AluOpType.mult
                            )
                            nc.gpsimd.tensor_tensor(
                                es[:, 512:640], es[:, 512:640], msk_sb[:], mybir.AluOpType.mult
                            )
                        es_ring[kt] = (es, qoff, W)

                    def emit_av(kt, p=p, qb=qb):
                        es, qoff, W = es_ring.pop(kt)
                        j0 = qoff // 128
                        for qc in range(j0, 4):
                            base = (qc - j0) * 128
                            for h_ in range(2):
                                es_chunk = es[:, h_ * 512 + base : h_ * 512 + base + 128]
                                _log(f"AV[{p},{qb}]kt{kt}q{qc}h{h_}")
                                # PSUM start=True zeroes the whole 2KB bank
                                # (the "zero region"), so exactly one start
                                # per yacc bank (chunks 0-1 / 2-3) per block:
                                # the first matmul into the bank at kt 0; one
                                # stop on the bank's last matmul.
                                nc.tensor.matmul(
                                    yacc_sb[:, qc * 256 + h_ * 65 : qc * 256 + h_ * 65 + 65],
                                    lhsT=es_chunk,
                                    rhs=va_sb[kt][:, (2 * p + h_) * 65 : (2 * p + h_ + 1) * 65],
                                    start=(kt == 0 and h_ == 0 and qc in (0, 2)),
                                    stop=(kt == 4 * qb + qc and h_ == 1 and qc in (1, 3)),
                                )

                    last_block = idx == len(blocks) - 1

                    def emit_norm(qc, p=p, qb=qb, last_block=last_block):
                        # softmax normalization for chunk qc, as soon as its
                        # accumulation group stops: per-partition reciprocal
                        # of the denominators (column 64 of each head), a
                        # tensor_scalar per head, then transpose yn [q, dA|dB]
                        # -> yt [d, q]. Normally the transpose rides the DMA
                        # crossbar; for the last block (whose yt feeds the
                        # epilogue immediately) it runs on the PE into the
                        # now-idle st banks to skip the ~2.5us DMA latency.
                        rec = p_sc.tile([128, 2], F32, tag="rec", name="rec", bufs=8)
                        yview = yacc_sb[:, qc * 256 : qc * 256 + 130].rearrange(
                            "p (g e) -> p g e", e=65
                        )
                        nc.vector.reciprocal(out=rec[:], in_=yview[:, :, 64:65])
                        dt_yn = F32 if last_block else BF16
                        yn = p_sc.tile([128, 128], dt_yn, tag="ynf" if last_block else "yn", name="yn", bufs=4 if last_block else 16)
                        for h_ in range(2):
                            nc.vector.tensor_scalar(
                                yn[:, h_ * 64 : (h_ + 1) * 64],
                                yacc_sb[:, qc * 256 + h_ * 65 : qc * 256 + h_ * 65 + 64],
                                rec[:, h_ : h_ + 1],
                                None,
                                mybir.AluOpType.mult,
                            )
                        ytd = yt_sb[p][:, (qb * 4 + qc) * 128 : (qb * 4 + qc + 1) * 128]
                        if last_block:
                            tp = p_ps.tile([128, 1024], F32, tag="st", name="sttp", bufs=2)
                            _log(f"tp[{qc}]")
                            nc.tensor.matmul(
                                tp[:, (qc % 2) * 512 : (qc % 2) * 512 + 128],
                                lhsT=yn[:],
                                rhs=idn_sb[:],
                                is_transpose=True,
                                start=True,
                                stop=True,
                            )
                            nc.vector.tensor_copy(
                                out=ytd, in_=tp[:, (qc % 2) * 512 : (qc % 2) * 512 + 128]
                            )
                        else:
                            nc.sync.dma_start_transpose(out=ytd, in_=yn[:])

                    total_iters = nkt + LAG
                    half0 = total_iters // 2
                    pops_done = 0
                    pops_b = 0
                    for kt in range(total_iters):
                        if kt < nkt:
                            emit_qk(kt)
                        if kt >= LAG:
                            emit_av(kt - LAG)
                            # norms wait for the owning yacc BANK's stop
                            # (whole-bank accumulation groups): bank 0
                            # (chunks 0-1) stops 3 iters before block end,
                            # bank 1 (chunks 2-3) at the last AV.
                            if kt == total_iters - 3:
                                emit_norm(0)
                            elif kt == total_iters - 2:
                                emit_norm(1)
                        want = (kt + 1) * len(filler) // total_iters
                        while pops_done < want:
                            filler[pops_done]()
                            pops_done += 1
                        if kt >= half0 and filler_b:
                            want_b = (kt + 1 - half0) * len(filler_b) // (total_iters - half0)
                            while pops_b < want_b:
                                filler_b[pops_b]()
                                pops_b += 1
                    emit_norm(2)
                    emit_norm(3)
                    while pops_done < len(filler):
                        filler[pops_done]()
                        pops_done += 1
                    while pops_b < len(filler_b):
                        filler_b[pops_b]()
                        pops_b += 1

                # epilogue: final projection group
                for tt in range(4 * (QB - 1), 4 * QB):
                    for cl in proj_closures(tt):
                        cl()

    nc.compile()
    return nc


_NC_CACHE = None


def _get_program():
    global _NC_CACHE
    if _NC_CACHE is None:
        _NC_CACHE = build_program()
    return _NC_CACHE


def _make_masks():
    ki = np.arange(128)[:, None]
    qi = np.arange(128)[None, :]
    return (ki <= qi).astype(np.float32).astype(ml_dtypes.bfloat16)


def make_in_maps(x, w_attn, w_proj):
    """Pack inputs partition-major (see build_program docstring) so each
    input lands in a handful of large contiguous DMAs."""
    msk = _make_masks()
    idn = np.eye(128, dtype=np.float32)
    xts, was, wps = [], [], []
    for b in range(B):
        xtT = np.ascontiguousarray(x[b].T)  # [C, T]
        xp = xtT.reshape(8, 128, 4, 512).transpose(1, 2, 0, 3).reshape(128, 16384)
        xts.append(xp.astype(ml_dtypes.bfloat16, order="C"))
    for g in range(2):
        q = [w_attn[:, 512 * g + 128 * p : 512 * g + 128 * (p + 1)] for p in range(4)]
        k = [w_attn[:, C + 512 * g + 128 * p : C + 512 * g + 128 * (p + 1)] for p in range(4)]
        v = w_attn[:, 2 * C + 512 * g : 2 * C + 512 * g + 512]
        front = np.concatenate([v, q[0], k[0]], axis=1)  # [C, 768]
        back = np.concatenate([q[1], k[1], q[2], k[2], q[3], k[3]], axis=1)
        wa_pack = np.concatenate(
            [
                front.reshape(8, 128, 768).transpose(1, 0, 2).reshape(128, 6144),
                back.reshape(8, 128, 768).transpose(1, 0, 2).reshape(128, 6144),
            ],
            axis=1,
        )
        was.append(wa_pack.astype(ml_dtypes.bfloat16, order="C"))
        wpg = w_proj[512 * g : 512 * g + 512, :]  # [512, C]
        wp_pack = wpg.reshape(4, 128, 1024).transpose(1, 0, 2).reshape(128, 4096)
        wps.append(wp_pack.astype(ml_dtypes.bfloat16, order="C"))
    return [
        {"xt": xts[c // 2], "wa": was[c % 2], "wp": wps[c % 2], "msk": msk, "idn": idn}
        for c in range(N_CORES)
    ]


def gather_output(results):
    out = np.empty((B, T, C), np.float32)
    for b in range(B):
        out[b] = results[2 * b]["outp"] + results[2 * b + 1]["outp"]
    return out


_RUNNER = None


def _make_cached_runner(nc):
    """jit the SPMD executable once so repeat kernel() calls skip the
    per-call retrace/recompile that run_bass_kernel_spmd pays."""
    import jax
    from jax.sharding import Mesh, PartitionSpec
    from jax.experimental.shard_map import shard_map
    from concourse import bass2jax

    bass2jax.install_neuronx_cc_hook()
    partition_name = nc.partition_id_tensor.name if nc.partition_id_tensor else None
    in_names, out_names, out_avals, zero_outs = [], [], [], []
    for alloc in nc.m.functions[0].allocations:
        if not isinstance(alloc, mybir.MemoryLocationSet):
            continue
        name = alloc.memorylocations[0].name
        if alloc.kind == "ExternalInput":
            if name != partition_name:
                in_names.append(name)
        elif alloc.kind == "ExternalOutput":
            shape = tuple(alloc.tensor_shape)
            dtype = mybir.dt.np(alloc.dtype)
            out_names.append(name)
            out_avals.append(jax.core.ShapedArray(shape, dtype))
            zero_outs.append(np.zeros(shape, dtype))
    n_params = len(in_names)
    n_outs = len(out_avals)
    all_in_names = in_names + out_names
    if partition_name is not None:
        all_in_names.append(partition_name)

    def _body(*args):
        operands = list(args)
        if partition_name is not None:
            operands.append(bass2jax.partition_id_tensor())
        return tuple(
            bass2jax._bass_exec_p.bind(
                *operands,
                out_avals=tuple(out_avals),
                in_names=tuple(all_in_names),
                out_names=tuple(out_names),
                lowering_input_output_aliases=(),
                sim_require_finite=True,
                sim_require_nnan=True,
                nc=nc,
            )
        )

    devices = jax.devices()[:N_CORES]
    mesh = Mesh(np.asarray(devices), ("core",))
    spec = PartitionSpec("core")
    sharded = jax.jit(
        shard_map(
            _body,
            mesh=mesh,
            in_specs=(spec,) * (n_params + n_outs),
            out_specs=(spec,) * n_outs,
            check_rep=False,
        ),
        donate_argnums=tuple(range(n_params, n_params + n_outs)),
        keep_unused=True,
    )

    def run(in_maps):
        concat_in = [
            np.concatenate([np.asarray(in_maps[c][nm]) for c in range(N_CORES)], 0)
            for nm in in_names
        ]
        zeros = [
            np.zeros((N_CORES * z.shape[0], *z.shape[1:]), z.dtype) for z in zero_outs
        ]
        outs = sharded(*concat_in, *zeros)
        return [
            {
                name: np.asarray(outs[i]).reshape(N_CORES, *out_avals[i].shape)[c]
                for i, name in enumerate(out_names)
            }
            for c in range(N_CORES)
        ]

    return run


def kernel(x, w_attn, w_proj):
    global _RUNNER
    x = np.asarray(x, dtype=np.float32)
    w_attn = np.asarray(w_attn, dtype=np.float32)
    w_proj = np.asarray(w_proj, dtype=np.float32)
    nc = _get_program()
    in_maps = make_in_maps(x, w_attn, w_proj)
    if _RUNNER is None:
        try:
            _RUNNER = _make_cached_runner(nc)
        except Exception:
            _RUNNER = None
        if _RUNNER is None:
            res = run_bass_kernel_spmd(nc, in_maps, core_ids=list(range(N_CORES)))
            return gather_output(res.results)
    try:
        return gather_output(_RUNNER(in_maps))
    except Exception:
        res = run_bass_kernel_spmd(nc, in_maps, core_ids=list(range(N_CORES)))
        return gather_output(res.results)


# revision 51
# speedup vs baseline: 1.1497x; 1.0330x over previous
"""Causal self-attention (B=4, T=2048, C=1024, H=16, D=64) on 8 trn2 NeuronCores.

Sharding: core c handles batch b = c//2 and head group g = c%2 (8 heads each).
Each core computes the qkv projection for its heads, causal flash attention,
and a partial output projection (its heads' rows of w_proj). The host sums the
two partials per batch.

Per-core kernel layout notes:
  - Host supplies x[b].T (so the contraction dim C lands on SBUF partitions),
    w_attn columns for its heads laid out as [qk_p0|qk_p1|qk_p2|qk_p3|v]
    (per-pair 128 q cols + 128 k cols, then the 512 v cols), w_proj rows, a
    causal diagonal mask tile, and a 128x128 identity (for PE transposes).
    All bf16; PSUM accumulation fp32.
  - Q^T/K^T are head-pair-packed: tile [128, T] = head A dims on partitions
    0-63, head B on 64-127, so the two K=64 QK matmuls of a pair use distinct
    PE row groups.
  - S^T tiles ([k, q] layout) get exp on ScalarE with the 1/sqrt(D) scale
    folded in. AV runs with the exp'd S^T chunk as the *stationary* operand
    (lhsT = es [128k, 128q], rhs = v [128k, 64]), so each AV matmul streams
    only 64 columns instead of up to 512 - half the PE time of the
    v-stationary form. A parallel N=1 matmul against a ones column
    accumulates the softmax denominator.
  - The denominator lands per-partition (q on partitions), so the softmax
    division is a native DVE tensor_scalar with a per-partition reciprocal -
    no PE broadcast matmuls. The normalized y [q, d] tiles are transposed
    back to [d, q] by the DMA crossbar (dma_start_transpose), which is nearly
    free on the otherwise-idle DMA engines.
  - Work is ordered V -> per-pair (QT/KT -> attention) -> projection, with
    all projection work interleaved into the attention kt loops as filler so
    the PE static order never runs dry while ScalarE paces exp.
  - PSUM is bank-granular: st ring 2x2 banks, yacc ring 2x1, one fill bank
    (manually double-buffered halves for the projection accumulators) and one
    denominator bank.
"""

import sys

sys.path.insert(0, "/opt/trn_rl_repo")

import numpy as np
import ml_dtypes

from concourse import bacc, mybir
import concourse.bass as bass
from concourse.tile import TileContext
from concourse.bass_utils import run_bass_kernel_spmd

B, T, C, H, D = 4, 2048, 1024, 16, 64
N_CORES = 8
HL = H // 2  # heads per core: 8
CL = HL * D  # local channels per core: 512
BF16 = mybir.dt.bfloat16
F32 = mybir.dt.float32
KT_TILES = C // 128  # 8 contraction tiles for the qkv projection
TT_TILES = T // 128  # 16 token tiles
QB = 4  # q blocks of 512


EMIT_LOG = []  # label per PE matmul emission, for trace attribution


def _log(label):
    EMIT_LOG.append(label)


def build_program(unroll=1, warm=18):
    """unroll>1 repeats the whole compute body (for steady-state timing)."""
    EMIT_LOG.clear()
    nc = bacc.Bacc("TRN2", target_bir_lowering=False, debug=False, num_devices=N_CORES)
    # Packed partition-major inputs (one SBUF-shaped row per partition) so
    # the whole input load is a handful of large byte-bound DMAs instead of
    # ~45 dispatches on the serial HWDGE queue.
    #   xt: col = qb*4096 + ki*512 + t      (token block qb, C-chunk ki)
    #   wa: front 8x768 = per-ki [v 512 | q0 128 | k0 128], then
    #       back  8x768 = per-ki [q1 128 | k1 128 | ... | q3 | k3]
    #   wp: col = ki*1024 + c
    xt = nc.dram_tensor("xt", [128, 4 * KT_TILES * 512], BF16, kind="ExternalInput")
    wa = nc.dram_tensor("wa", [128, KT_TILES * 1536], BF16, kind="ExternalInput")
    wp = nc.dram_tensor("wp", [128, 4 * 1024], BF16, kind="ExternalInput")
    msk = nc.dram_tensor("msk", [128, 128], BF16, kind="ExternalInput")
    idn = nc.dram_tensor("idn", [128, 128], BF16, kind="ExternalInput")
    outp = nc.dram_tensor("outp", [T, C], F32, kind="ExternalOutput")

    with TileContext(nc) as tc:
        with tc.tile_pool(name="resid", bufs=1) as p_r, tc.tile_pool(
            name="ps", bufs=1, space="PSUM"
        ) as p_ps, tc.tile_pool(name="es", bufs=3) as p_es, tc.tile_pool(
            name="sc", bufs=4
        ) as p_sc, tc.tile_pool(name="ob", bufs=4) as p_ob:
            xt_sb = p_r.tile([128, 4 * KT_TILES * 512], BF16, name="xts", tag="xts")
            wa_sb = p_r.tile([128, KT_TILES * 1536], BF16, name="was", tag="was")
            wp_sb = p_r.tile([128, 4 * 1024], BF16, name="wps", tag="wps")

            def xt_ap(ki, qb, a, b):
                # token columns [qb*512+a, qb*512+b) of C-chunk ki
                return xt_sb[:, qb * 4096 + ki * 512 + a : qb * 4096 + ki * 512 + b]

            def wav_ap(ki, a, b):
                return wa_sb[:, ki * 768 + a : ki * 768 + b]

            def waqk_ap(ki, p, k_not_q):
                if p == 0:
                    c0 = ki * 768 + 512 + k_not_q * 128
                else:
                    c0 = 6144 + ki * 768 + (p - 1) * 256 + k_not_q * 128
                return wa_sb[:, c0 : c0 + 128]
            msk_sb = p_r.tile([128, 128], BF16, tag="msk")
            idn_sb = p_r.tile([128, 128], BF16, tag="idn")
            ones_sb = p_r.tile([128, 1], BF16, tag="ones")
            qt_sb = [p_r.tile([128, T], BF16, name=f"qt{p}", tag=f"qt{p}") for p in range(4)]
            kt_sb = [p_r.tile([128, T], BF16, name=f"kt{p}", tag=f"kt{p}") for p in range(4)]
            va_sb = [p_r.tile([128, HL * 65], BF16, name=f"va{i}", tag=f"va{i}") for i in range(TT_TILES)]
            yt_sb = [p_r.tile([128, T], BF16, name=f"yt{p}", tag=f"yt{p}") for p in range(4)]

            nc.sync.dma_start(out=msk_sb[:], in_=msk[:])
            nc.sync.dma_start(out=idn_sb[:], in_=idn[:])
            nc.gpsimd.memset(ones_sb[:], 1.0)

            # HAM warmup: the PE clock-gate sits at 1.2 GHz until ~3us of
            # sustained activity. The input DMA ramp leaves the PE idle at
            # kernel start, so burn it with dummy matmuls on a memset tile
            # (borrowing a "yacc" PSUM slot, which real work doesn't need
            # until the first attention block).
            warmt = p_r.tile([128, 512], BF16, tag="warm")
            nc.gpsimd.memset(warmt[:], 0.0)
            wps = p_ps.tile([128, 1024], F32, tag="st", name="warmps", bufs=2)
            for _ in range(warm):
                _log("warm")
                nc.tensor.matmul(
                    wps[:, 0:512], lhsT=warmt[:, 0:128], rhs=warmt[:], start=True, stop=True
                )
            # PSUM is bank-granular (8 x 2KB): st ring 2x2 banks, fill ring
            # 2x1, and one wide single-buffered yacc (2 banks) holding the
            # per-chunk [y_A 65 | y_B 65] AV accumulators, denominator riding
            # in column 64 via the ones column interleaved into va.
            yacc_sb = p_ps.tile([128, 1024], F32, tag="yacc", name="yacc", bufs=1)

            def fill_slot():
                return p_ps.tile([128, 512], F32, tag="fill", name="fill", bufs=2)[:]

            # ones columns of every va tile (written once; the v-projection
            # copies write through a 65-stride view that skips them)
            for tt in range(TT_TILES):
                va_view = va_sb[tt].rearrange("p (h e) -> p h e", e=65)
                nc.gpsimd.memset(va_view[:, :, 64:65], 1.0)
            # Input DMAs, ordered so the prologue (V tiles 0-3 + pair-0
            # QT/KT for q block 0) is fed first, in a handful of large
            # transfers: wa front (v + pair-0 qk) and xt token-block 0 split
            # in two for finer chasing, then the remaining xt blocks, the
            # other pairs' qk columns, and wp.
            for h in range(2):
                nc.sync.dma_start(
                    out=wa_sb[:, h * 3072 : (h + 1) * 3072],
                    in_=wa[:, h * 3072 : (h + 1) * 3072],
                )
                nc.sync.dma_start(
                    out=xt_sb[:, h * 2048 : (h + 1) * 2048],
                    in_=xt[:, h * 2048 : (h + 1) * 2048],
                )
            for qb in range(1, QB):
                nc.sync.dma_start(
                    out=xt_sb[:, qb * 4096 : (qb + 1) * 4096],
                    in_=xt[:, qb * 4096 : (qb + 1) * 4096],
                )
            nc.sync.dma_start(out=wa_sb[:, 6144:12288], in_=wa[:, 6144:12288])
            nc.sync.dma_start(out=wp_sb[:], in_=wp[:])

            for _ in range(unroll):
                # ---- closure builders; each closure is a chunk of PE work
                # that can be interleaved into the attention kt loops so the
                # PE static order never runs dry while ScalarE paces exp.
                def v_closures(tt):
                    def h(tt=tt):
                        acc = fill_slot()
                        for ki in range(KT_TILES):
                            _log(f"v[{tt}]k{ki}")
                            nc.tensor.matmul(
                                acc,
                                lhsT=xt_ap(ki, tt // 4, (tt % 4) * 128, (tt % 4) * 128 + 128),
                                rhs=wav_ap(ki, 0, 512),
                                start=(ki == 0),
                                stop=(ki == KT_TILES - 1),
                            )
                        va_view = va_sb[tt].rearrange("p (h e) -> p h e", e=65)
                        nc.vector.tensor_copy(
                            out=va_view[:, :, 0:64],
                            in_=acc.rearrange("p (h e) -> p h e", e=64),
                        )
                    return [h]

                def accqk_closures(p, qb):
                    out = []
                    for dst, knq in ((qt_sb, 0), (kt_sb, 1)):
                        def h(dst=dst, knq=knq, p=p, qb=qb):
                            acc = fill_slot()
                            for ki in range(KT_TILES):
                                _log(f"qk[{p},{qb},{knq}]k{ki}")
                                nc.tensor.matmul(
                                    acc,
                                    lhsT=waqk_ap(ki, p, knq),
                                    rhs=xt_ap(ki, qb, 0, 512),
                                    start=(ki == 0),
                                    stop=(ki == KT_TILES - 1),
                                )
                            nc.vector.tensor_copy(
                                out=dst[p][:, qb * 512 : (qb + 1) * 512],
                                in_=acc,
                            )
                        out.append(h)
                    return out

                def proj_closures(tt):
                    out = []
                    state = {}

                    def get_ob():
                        if "ob" not in state:
                            state["ob"] = p_ob.tile([128, C], F32, tag="ob", name="ob")
                        return state["ob"]

                    for nb in range(2):
                        def h(tt=tt, nb=nb):
                            acc = fill_slot()
                            for pp in range(4):
                                _log(f"proj[{tt},{nb}]p{pp}")
                                nc.tensor.matmul(
                                    acc,
                                    lhsT=yt_sb[pp][:, tt * 128 : (tt + 1) * 128],
                                    rhs=wp_sb[:, pp * 1024 + nb * 512 : pp * 1024 + (nb + 1) * 512],
                                    start=(pp == 0),
                                    stop=(pp == 3),
                                )
                            ob = get_ob()
                            nc.vector.tensor_copy(
                                out=ob[:, nb * 512 : (nb + 1) * 512], in_=acc
                            )
                            if nb == 1:
                                nc.sync.dma_start(
                                    out=outp[tt * 128 : (tt + 1) * 128, :], in_=ob[:]
                                )
                        out.append(h)
                    return out

                blocks = [(p, qb) for p in range(4) for qb in range(QB)]

                # prologue: V tiles for q block 0 and pair 0's first QT/KT
                for tt in range(4):
                    for cl in v_closures(tt):
                        cl()
                for cl in accqk_closures(0, 0):
                    cl()

                for idx, (p, qb) in enumerate(blocks):
                    filler = []
                    filler_b = []
                    if idx + 1 < len(blocks):
                        filler += accqk_closures(*blocks[idx + 1])
                    if p == 0 and qb < QB - 1:
                        for tt in range(4 * (qb + 1), 4 * (qb + 1) + 4):
                            filler += v_closures(tt)
                    if p == 3 and qb > 0:
                        # proj consumes the previous block's transposes; pop it
                        # only in the second half of the block so the DMA
                        # crossbar has drained by then.
                        for tt in range(4 * (qb - 1), 4 * qb):
                            filler_b += proj_closures(tt)

                    nkt = 4 * qb + 4
                    LAG = 3  # AV trails QK by this many k tiles
                    es_ring = {}

                    def emit_qk(kt, p=p, qb=qb):
                        # Diagonal tiles only need q columns >= kt*128; the
                        # causal triangle then only lives in the first 128 of
                        # the remaining columns. st/es keep head B at column
                        # 512 (fixed full-size tiles).
                        j = kt - 4 * qb
                        qoff = max(j, 0) * 128
                        W = 512 - qoff
                        st = p_ps.tile([128, 1024], F32, tag="st", name="st", bufs=2)
                        _log(f"QK[{p},{qb}]kt{kt}A")
                        nc.tensor.matmul(
                            st[:, 0:W],
                            lhsT=kt_sb[p][0:64, kt * 128 : (kt + 1) * 128],
                            rhs=qt_sb[p][0:64, qb * 512 + qoff : (qb + 1) * 512],
                            start=True,
                            stop=True,
                        )
                        _log(f"QK[{p},{qb}]kt{kt}B")
                        nc.tensor.matmul(
                            st[:, 512 : 512 + W],
                            lhsT=kt_sb[p][64:128, kt * 128 : (kt + 1) * 128],
                            rhs=qt_sb[p][64:128, qb * 512 + qoff : (qb + 1) * 512],
                            start=True,
                            stop=True,
                        )
                        es = p_es.tile([128, 1024], BF16, tag="es", name="es", bufs=6)
                        if W == 512:
                            nc.scalar.activation(
                                out=es[:],
                                in_=st[:],
                                func=mybir.ActivationFunctionType.Exp,
                                scale=0.125,
                            )
                        else:
                            view_es = es.rearrange("p (h w) -> p h w", h=2)[:, :, 0:W]
                            view_st = st.rearrange("p (h w) -> p h w", h=2)[:, :, 0:W]
                            nc.scalar.activation(
                                out=view_es,
                                in_=view_st,
                                func=mybir.ActivationFunctionType.Exp,
                                scale=0.125,
                            )
                        if j >= 0:
                            nc.gpsimd.tensor_tensor(
                                es[:, 0:128], es[:, 0:128], msk_sb[:], mybir.AluOpType.mult
                            )
                            nc.gpsimd.tensor_tensor(
                                es[:, 512:640], es[:, 512:640], msk_sb[:], mybir.AluOpType.mult
                            )
                        es_ring[kt] = (es, qoff, W)

                    def emit_av(kt, p=p, qb=qb):
                        es, qoff, W = es_ring.pop(kt)
                        j0 = qoff // 128
                        for qc in range(j0, 4):
                            base = (qc - j0) * 128
                            for h_ in range(2):
                                es_chunk = es[:, h_ * 512 + base : h_ * 512 + base + 128]
                                _log(f"AV[{p},{qb}]kt{kt}q{qc}h{h_}")
                                # PSUM start=True zeroes the whole 2KB bank
                                # (the "zero region"), so exactly one start
                                # per yacc bank (chunks 0-1 / 2-3) per block:
                                # the first matmul into the bank at kt 0; one
                                # stop on the bank's last matmul.
                                nc.tensor.matmul(
                                    yacc_sb[:, qc * 256 + h_ * 65 : qc * 256 + h_ * 65 + 65],
                                    lhsT=es_chunk,
                                    rhs=va_sb[kt][:, (2 * p + h_) * 65 : (2 * p + h_ + 1) * 65],
                                    start=(kt == 0 and h_ == 0 and qc in (0, 2)),
                                    stop=(kt == 4 * qb + qc and h_ == 1 and qc in (1, 3)),
                                )

                    # pair 3's yt tiles feed proj fillers in the very next
                    # block; route their transposes over the PE (st banks,
                    # emitted after the block's QKs) instead of the DMA
                    # crossbar whose ~2.5us completion would stall those
                    # proj matmuls.
                    last_block = p == 3

                    def emit_norm(qc, p=p, qb=qb, last_block=last_block):
                        # softmax normalization for chunk qc once its yacc
                        # bank's accumulation group has stopped: per-partition
                        # reciprocal of the denominators (column 64 of each
                        # head), a tensor_scalar per head, then transpose
                        # yn [q, dA|dB] -> yt [d, q]. Normally the transpose
                        # rides the DMA crossbar; for the last block (whose yt
                        # feeds the epilogue immediately) it runs on the PE
                        # into the now-idle st banks to skip the DMA latency.
                        rec = p_sc.tile([128, 2], F32, tag="rec", name="rec", bufs=8)
                        yview = yacc_sb[:, qc * 256 : qc * 256 + 130].rearrange(
                            "p (g e) -> p g e", e=65
                        )
                        nc.vector.reciprocal(out=rec[:], in_=yview[:, :, 64:65])
                        yn = p_sc.tile([128, 128], BF16, tag="yn", name="yn", bufs=16)
                        for h_ in range(2):
                            nc.vector.tensor_scalar(
                                yn[:, h_ * 64 : (h_ + 1) * 64],
                                yacc_sb[:, qc * 256 + h_ * 65 : qc * 256 + h_ * 65 + 64],
                                rec[:, h_ : h_ + 1],
                                None,
                                mybir.AluOpType.mult,
                            )
                        ytd = yt_sb[p][:, (qb * 4 + qc) * 128 : (qb * 4 + qc + 1) * 128]
                        if last_block:
                            tp = p_ps.tile([128, 1024], F32, tag="st", name="sttp", bufs=2)
                            tpb = tp.bitcast(BF16)
                            _log(f"tp[{qc}]")
                            nc.tensor.matmul(
                                tpb[:, (qc % 2) * 1024 : (qc % 2) * 1024 + 128],
                                lhsT=yn[:],
                                rhs=idn_sb[:],
                                is_transpose=True,
                                start=True,
                                stop=True,
                            )
                            nc.vector.tensor_copy(
                                out=ytd, in_=tpb[:, (qc % 2) * 1024 : (qc % 2) * 1024 + 128]
                            )
                        else:
                            nc.sync.dma_start_transpose(out=ytd, in_=yn[:])

                    total_iters = nkt + LAG
                    half0 = total_iters // 2
                    pops_done = 0
                    pops_b = 0
                    for kt in range(total_iters):
                        if kt < nkt:
                            emit_qk(kt)
                        if kt >= LAG:
                            emit_av(kt - LAG)
                            # norms wait for the owning yacc BANK's stop
                            # (whole-bank accumulation groups): bank 0
                            # (chunks 0-1) stops 3 iters before block end,
                            # bank 1 (chunks 2-3) at the last AV.
                            if kt == total_iters - 3:
                                emit_norm(0)
                            elif kt == total_iters - 2:
                                emit_norm(1)
                        want = (kt + 1) * len(filler) // total_iters
                        while pops_done < want:
                            filler[pops_done]()
                            pops_done += 1
                        if kt >= half0 and filler_b:
                            want_b = (kt + 1 - half0) * len(filler_b) // (total_iters - half0)
                            while pops_b < want_b:
                                filler_b[pops_b]()
                                pops_b += 1
                    emit_norm(2)
                    emit_norm(3)
                    while pops_done < len(filler):
                        filler[pops_done]()
                        pops_done += 1
                    while pops_b < len(filler_b):
                        filler_b[pops_b]()
                        pops_b += 1

                # epilogue: final projection group
                for tt in range(4 * (QB - 1), 4 * QB):
                    for cl in proj_closures(tt):
                        cl()

    nc.compile()
    return nc


_NC_CACHE = None


def _get_program():
    global _NC_CACHE
    if _NC_CACHE is None:
        _NC_CACHE = build_program()
    return _NC_CACHE


def _make_masks():
    ki = np.arange(128)[:, None]
    qi = np.arange(128)[None, :]
    return (ki <= qi).astype(np.float32).astype(ml_dtypes.bfloat16)


def make_in_maps(x, w_attn, w_proj):
    """Pack inputs partition-major (see build_program docstring) so each
    input lands in a handful of large contiguous DMAs."""
    msk = _make_masks()
    idn = np.eye(128, dtype=np.float32).astype(ml_dtypes.bfloat16)
    xts, was, wps = [], [], []
    for b in range(B):
        xtT = np.ascontiguousarray(x[b].T)  # [C, T]
        xp = xtT.reshape(8, 128, 4, 512).transpose(1, 2, 0, 3).reshape(128, 16384)
        xts.append(xp.astype(ml_dtypes.bfloat16, order="C"))
    for g in range(2):
        q = [w_attn[:, 512 * g + 128 * p : 512 * g + 128 * (p + 1)] for p in range(4)]
        k = [w_attn[:, C + 512 * g + 128 * p : C + 512 * g + 128 * (p + 1)] for p in range(4)]
        v = w_attn[:, 2 * C + 512 * g : 2 * C + 512 * g + 512]
        front = np.concatenate([v, q[0], k[0]], axis=1)  # [C, 768]
        back = np.concatenate([q[1], k[1], q[2], k[2], q[3], k[3]], axis=1)
        wa_pack = np.concatenate(
            [
                front.reshape(8, 128, 768).transpose(1, 0, 2).reshape(128, 6144),
                back.reshape(8, 128, 768).transpose(1, 0, 2).reshape(128, 6144),
            ],
            axis=1,
        )
        was.append(wa_pack.astype(ml_dtypes.bfloat16, order="C"))
        wpg = w_proj[512 * g : 512 * g + 512, :]  # [512, C]
        wp_pack = wpg.reshape(4, 128, 1024).transpose(1, 0, 2).reshape(128, 4096)
        wps.append(wp_pack.astype(ml_dtypes.bfloat16, order="C"))
    return [
        {"xt": xts[c // 2], "wa": was[c % 2], "wp": wps[c % 2], "msk": msk, "idn": idn}
        for c in range(N_CORES)
    ]


def gather_output(results):
    out = np.empty((B, T, C), np.float32)
    for b in range(B):
        out[b] = results[2 * b]["outp"] + results[2 * b + 1]["outp"]
    return out


_RUNNER = None


def _make_cached_runner(nc):
    """jit the SPMD executable once so repeat kernel() calls skip the
    per-call retrace/recompile that run_bass_kernel_spmd pays."""
    import jax
    from jax.sharding import Mesh, PartitionSpec
    from jax.experimental.shard_map import shard_map
    from concourse import bass2jax

    bass2jax.install_neuronx_cc_hook()
    partition_name = nc.partition_id_tensor.name if nc.partition_id_tensor else None
    in_names, out_names, out_avals, zero_outs = [], [], [], []
    for alloc in nc.m.functions[0].allocations:
        if not isinstance(alloc, mybir.MemoryLocationSet):
            continue
        name = alloc.memorylocations[0].name
        if alloc.kind == "ExternalInput":
            if name != partition_name:
                in_names.append(name)
        elif alloc.kind == "ExternalOutput":
            shape = tuple(alloc.tensor_shape)
            dtype = mybir.dt.np(alloc.dtype)
            out_names.append(name)
            out_avals.append(jax.core.ShapedArray(shape, dtype))
            zero_outs.append(np.zeros(shape, dtype))
    n_params = len(in_names)
    n_outs = len(out_avals)
    all_in_names = in_names + out_names
    if partition_name is not None:
        all_in_names.append(partition_name)

    def _body(*args):
        operands = list(args)
        if partition_name is not None:
            operands.append(bass2jax.partition_id_tensor())
        return tuple(
            bass2jax._bass_exec_p.bind(
                *operands,
                out_avals=tuple(out_avals),
                in_names=tuple(all_in_names),
                out_names=tuple(out_names),
                lowering_input_output_aliases=(),
                sim_require_finite=True,
                sim_require_nnan=True,
                nc=nc,
            )
        )

    devices = jax.devices()[:N_CORES]
    mesh = Mesh(np.asarray(devices), ("core",))
    spec = PartitionSpec("core")
    sharded = jax.jit(
        shard_map(
            _body,
            mesh=mesh,
            in_specs=(spec,) * (n_params + n_outs),
            out_specs=(spec,) * n_outs,
            check_rep=False,
        ),
        donate_argnums=tuple(range(n_params, n_params + n_outs)),
        keep_unused=True,
    )

    def run(in_maps):
        concat_in = [
            np.concatenate([np.asarray(in_maps[c][nm]) for c in range(N_CORES)], 0)
            for nm in in_names
        ]
        zeros = [
            np.zeros((N_CORES * z.shape[0], *z.shape[1:]), z.dtype) for z in zero_outs
        ]
        outs = sharded(*concat_in, *zeros)
        return [
            {
                name: np.asarray(outs[i]).reshape(N_CORES, *out_avals[i].shape)[c]
                for i, name in enumerate(out_names)
            }
            for c in range(N_CORES)
        ]

    return run


def kernel(x, w_attn, w_proj):
    global _RUNNER
    x = np.asarray(x, dtype=np.float32)
    w_attn = np.asarray(w_attn, dtype=np.float32)
    w_proj = np.asarray(w_proj, dtype=np.float32)
    nc = _get_program()
    in_maps = make_in_maps(x, w_attn, w_proj)
    if _RUNNER is None:
        try:
            _RUNNER = _make_cached_runner(nc)
        except Exception:
            _RUNNER = None
        if _RUNNER is None:
            res = run_bass_kernel_spmd(nc, in_maps, core_ids=list(range(N_CORES)))
            return gather_output(res.results)
    try:
        return gather_output(_RUNNER(in_maps))
    except Exception:
        res = run_bass_kernel_spmd(nc, in_maps, core_ids=list(range(N_CORES)))
        return gather_output(res.results)
